# revision 28
# baseline (speedup 1.0000x reference)
"""Trainium2 Bass kernel for nn_MessagePassing_42588895707817.

out = (h @ W.T + b) @ norm_graph,  norm_graph = graph / clip(rowsum(graph), EPS)

Math folding: out = h @ C + 1*d  with  C = W.T @ norm_graph  (128x128),
d = b @ norm_graph (b is zeros for this problem; general path kept).

Strategy (memory-bound => minimize HBM bytes):
- Host pre-stages h transposed and cast to fp16: ht[i] = h_core_i.T
  [128 f, 32768 tok].  Device reads HALF the f32 bytes and the tile is
  already in matmul-rhs layout (f on partitions) -- no PE transpose, no
  PSUM->SBUF staging of inputs at all.
- C is computed on device in f32 (graph row-norm + one matmul), cast to
  fp16 once, and used as the STATIONARY lhsT for every matmul:
      psum[g, t] = sum_f C[f,g] * ht[f,t]   (= out^T tile, 512 tok wide)
- DVE drains each PSUM bank with a cast-copy f32->fp16 into the out^T
  SBUF chunk; chunk DMAs back to HBM as fp16 (again half the bytes).
  Host transposes/upcasts to the full [32,8192,128] f32 result.

Sharding: data-parallel on batch B=32 across 8 cores (4 batches/core).
Per-core HBM traffic: 8 MiB in + 8 MiB out (fp16) vs 33.5 MiB for f32.
fp16 rounding contributes ~5e-4 rel err vs the 2e-2 gate.
"""

import sys

if "/opt/trn_rl_repo" not in sys.path:
    sys.path.insert(0, "/opt/trn_rl_repo")

from contextlib import ExitStack

import numpy as np

B, T, FDIM, HID = 32, 8192, 128, 128
EPS = 1e-10
NCORES = 8
B_LOC = B // NCORES
NTOK = B_LOC * T  # 32768 tokens per core

P = 128   # SBUF partitions
MM = 512  # moving free dim per matmul = one PSUM bank of f32


def build_program(ntok=NTOK, cht=4096, b_nonzero=False, copy_mix="vs",
                  mm=MM, ps_bufs=5, out_q="g", drain_banks=1, ld_bufs=3,
                  warm_every=2, tail_split=2, st_bufs=3, out_split=2,
                  head_split=True):
    """copy_mix: engines for the PSUM->SBUF drains, cycled per drain:
    'v'=vector, 's'=scalar (gpsimd can NOT read PSUM).
    mm: moving free dim per matmul (512 f32 = one PSUM bank, ISA max).
    out_q: engine for output DMA doorbells. Only sync ('y') and scalar
    ('s') have hardware DGE rings; gpsimd ('g') is software-DGE with
    ~4us doorbell-to-data latency but zero contention with the drains.
    Don't share the sync ring with input (serializes the streams).
    drain_banks: PSUM banks per drain copy (amortizes per-op sem tax).
    ld_bufs: input chunk buffers. Do NOT prefetch everything: SDMA
    engines drain a whole queue backlog before round-robining to the
    output queue, which starves the output stream.
    warm_every: insert a short dummy matmul (scratch PSUM, no reader)
    after every N real matmuls to keep the PE HAM clock-gate at 8/8
    (2.4 GHz); 0 disables.
    tail_split: split the final chunk into this many sub-chunks, with
    doorbells on the (by then idle) scalar HWDGE ring, shortening the
    drain->store tail."""
    import concourse.bacc as bacc
    import concourse.tile as tile
    from concourse import mybir

    f32 = mybir.dt.float32
    f16 = mybir.dt.float16
    nchunks = ntok // cht
    assert ntok % cht == 0 and cht % (mm * drain_banks) == 0
    assert cht % (mm * drain_banks * max(tail_split, 1)) == 0
    psum_used = ps_bufs * mm * drain_banks + 512 + (512 if warm_every else 0)
    assert psum_used <= 4096, f"PSUM overflow: {psum_used}"  # 8 banks x 512 f32

    nc = bacc.Bacc("TRN2", debug=False, target_bir_lowering=False)

    ht_d = nc.dram_tensor("ht", [P, ntok], f16, kind="ExternalInput")
    graph_d = nc.dram_tensor("graph", [FDIM, FDIM], f32, kind="ExternalInput")
    w_d = nc.dram_tensor("W", [HID, FDIM], f32, kind="ExternalInput")
    b_d = nc.dram_tensor("b", [1, HID], f32, kind="ExternalInput")
    ot_d = nc.dram_tensor("ot", [P, ntok], f16, kind="ExternalOutput")

    # chunk plan: (view, view-index, tokens, out-doorbell engine key)
    tsp = max(tail_split, 1)
    fine = cht // tsp
    assert out_split in (1, tsp), "out_split reuses the fine view"
    ht_v = ht_d[:].rearrange("f (c t) -> c f t", t=cht)
    ot_v = ot_d[:].rearrange("g (c t) -> c g t", t=cht)
    ht_w = ht_d[:].rearrange("f (c t) -> c f t", t=fine)
    ot_w = ot_d[:].rearrange("g (c t) -> c g t", t=fine)
    # head chunk split fine: PE starts on a quarter-filled pipe, so first
    # drains (and the output stream) exist ~2us earlier
    if head_split:
        plan = [("f", k, fine, out_q) for k in range(tsp)]
        plan += [("m", c, cht, out_q) for c in range(1, nchunks - 1)]
    else:
        plan = [("m", c, cht, out_q) for c in range(nchunks - 1)]
    plan += [("f", (nchunks - 1) * tsp + k, fine, "s") for k in range(tsp)]

    with tile.TileContext(nc) as tc, ExitStack() as ctx:
        singles = ctx.enter_context(tc.tile_pool(name="singles", bufs=1))
        ld = ctx.enter_context(tc.tile_pool(name="ld", bufs=ld_bufs))
        st = ctx.enter_context(tc.tile_pool(name="st", bufs=st_bufs))
        ps = ctx.enter_context(tc.tile_pool(name="ps", bufs=ps_bufs, space="PSUM"))
        ps_pre = ctx.enter_context(tc.tile_pool(name="ps_pre", bufs=1, space="PSUM"))

        graph_s = singles.tile([P, P], f32)
        nc.sync.dma_start(out=graph_s, in_=graph_d[:])
        w_raw = singles.tile([P, P], f32)
        nc.sync.dma_start(out=w_raw, in_=w_d[:])

        # Stage W through DVE so the preamble matmul depends on DVE only.
        w_s = singles.tile([P, P], f32)
        nc.vector.tensor_copy(w_s, w_raw)

        # norm_graph = graph / max(rowsum(graph), EPS)
        deg = singles.tile([P, 1], f32)
        nc.vector.tensor_reduce(deg, graph_s, axis=mybir.AxisListType.X,
                                op=mybir.AluOpType.add)
        nc.vector.tensor_scalar_max(deg, deg, EPS)
        rdeg = singles.tile([P, 1], f32)
        nc.vector.reciprocal(rdeg, deg)
        norm_s = singles.tile([P, P], f32)
        nc.vector.tensor_scalar_mul(norm_s, graph_s, rdeg)

        # C = W.T @ norm_graph   [f, g], cast fp16 for the streaming matmuls
        c_ps = ps_pre.tile([P, P], f32, tag="pre")
        nc.tensor.matmul(c_ps, lhsT=w_s, rhs=norm_s, start=True, stop=True)
        c_s = singles.tile([P, P], f16)
        nc.vector.tensor_copy(c_s, c_ps)

        if b_nonzero:
            # d = b @ norm_graph as [1, g]; psum pre-fill via ones rhs
            b_raw = singles.tile([P, 1], f32)
            nc.sync.dma_start(out=b_raw, in_=b_d[:].rearrange("o h -> h o"))
            b_col = singles.tile([P, 1], f32)
            nc.vector.tensor_copy(b_col, b_raw)
            d_ps = ps_pre.tile([1, P], f32, tag="pre")
            nc.tensor.matmul(d_ps, lhsT=b_col, rhs=norm_s, start=True, stop=True)
            d_s = singles.tile([1, P], f16)
            nc.vector.tensor_copy(d_s, d_ps)
            ones_s = singles.tile([1, mm], f16)
            nc.vector.memset(ones_s, 1.0)

        eng = {"v": nc.vector.tensor_copy, "s": nc.scalar.copy}
        dma_q = {"g": nc.gpsimd.dma_start, "s": nc.scalar.dma_start,
                 "y": nc.sync.dma_start}
        dwid = mm * drain_banks
        if warm_every:
            warm_ps = ctx.enter_context(
                tc.tile_pool(name="warm", bufs=1, space="PSUM"))
            scratch = warm_ps.tile([P, P], f32, tag="scratch")
        n_mm = 0
        n_dr = 0
        for kind, ci, tokens, oq in plan:
            src = ht_v[ci] if kind == "m" else ht_w[ci]
            dst = ot_v[ci] if kind == "m" else ot_w[ci]
            in_t = ld.tile([P, tokens], f16)
            nc.sync.dma_start(out=in_t, in_=src)
            out_t = st.tile([P, tokens], f16)
            for g in range(tokens // dwid):
                ps_t = ps.tile([P, dwid], f32)
                for j in range(drain_banks):
                    sl = slice(j * mm, (j + 1) * mm)
                    rl = slice(g * dwid + j * mm, g * dwid + (j + 1) * mm)
                    if b_nonzero:
                        nc.tensor.matmul(ps_t[:, sl], lhsT=d_s, rhs=ones_s,
                                         start=True, stop=False)
                        nc.tensor.matmul(ps_t[:, sl], lhsT=c_s, rhs=in_t[:, rl],
                                         start=False, stop=True)
                    else:
                        nc.tensor.matmul(ps_t[:, sl], lhsT=c_s, rhs=in_t[:, rl],
                                         start=True, stop=True)
                    n_mm += 1
                    if warm_every and n_mm % warm_every == 0:
                        # PE busywork into a never-read scratch bank: absorbs
                        # pipeline micro-idles so the HAM clock-gate stays 8/8
                        nc.tensor.matmul(scratch, lhsT=c_s, rhs=c_s,
                                         start=True, stop=True,
                                         skip_group_check=True)
                eng[copy_mix[n_dr % len(copy_mix)]](
                    out_t[:, g * dwid:(g + 1) * dwid], ps_t)
                n_dr += 1
                # ship finished half-chunks early: drained bytes reach the
                # SDMA engines sooner, so they never starve for output work
                if kind == "m" and out_split > 1 and \
                        (g + 1) % (tokens // dwid // out_split) == 0:
                    h_ix = (g + 1) // (tokens // dwid // out_split) - 1
                    dma_q[oq](out=ot_w[ci * tsp + h_ix],
                              in_=out_t[:, h_ix * fine:(h_ix + 1) * fine])
            if kind != "m" or out_split == 1:
                dma_q[oq](out=dst, in_=out_t)

    nc.compile()
    return nc


def make_in_maps(h, graph, W, b):
    b2 = np.ascontiguousarray(b, dtype=np.float32).reshape(1, HID)
    graph = np.ascontiguousarray(graph, dtype=np.float32)
    W = np.ascontiguousarray(W, dtype=np.float32)
    hs = np.asarray(h, dtype=np.float32).reshape(NCORES, NTOK, FDIM)
    return [
        {"ht": hs[i].T.astype(np.float16), "graph": graph, "W": W, "b": b2}
        for i in range(NCORES)
    ]


def unshard_out(res):
    outs = []
    for i in range(NCORES):
        ot = res.results[i]["ot"]  # [128 g, 32768 tok] fp16
        outs.append(ot.reshape(HID, B_LOC, T).transpose(1, 2, 0))
    return np.concatenate(outs, axis=0).astype(np.float32)


def kernel(h, graph, W, b):
    # NOTE: walrus --enable-ldw-opt=true is NOT usable here: 16-bit matmuls
    # lower to standalone InstLdweights, which that optimization rejects.
    from concourse import bass_utils

    nc = build_program(b_nonzero=bool(np.any(np.asarray(b))))
    in_maps = make_in_maps(h, graph, W, b)
    res = bass_utils.run_bass_kernel_spmd(nc, in_maps, list(range(NCORES)))
    return unshard_out(res)
